# revision 3
# baseline (speedup 1.0000x reference)
"""GAT-style attention block on 8 TRN2 NeuronCores.

Math (mirrors the reference):
  HW = features @ W                     [N, 128]
  s = HW @ a[:128],  t = HW @ a[128:]   [N]
  e_ij = leaky_relu(s_i + t_j) * adj_ij        (leaky commutes with adj >= 0)
  cn_j = sqrt(sum_i e_ij^2)
  out = (e / max(cn, 1e-12)) @ HW

Sharding: rows of e across 8 cores ([N/8, N] slab each, stored transposed
with j on partitions); column-norm partials AllReduced (32 KB).
"""

import numpy as np
import ml_dtypes
from contextlib import ExitStack

import concourse.bass as bass
import concourse.bacc as bacc
import concourse.tile as tile
from concourse import mybir
from concourse import bass_utils

BF16 = mybir.dt.bfloat16
F32 = mybir.dt.float32
U8 = mybir.dt.uint8

N = 8192
N_CORES = 8
F_IN = 256
F_OUT = 128
NEG_SLOPE = 0.2
NORM_EPS = 1e-12

# ---------------------------------------------------------------- custom DVE op
_EDGE_OP = None


def _register_edge_op():
    """e = in0 * leaky_relu(in1 + s0); leaky via 0.2u + 0.8*relu(u)."""
    global _EDGE_OP
    if _EDGE_OP is not None:
        return _EDGE_OP
    from concourse import dve_ops
    from concourse.dve_spec import Spec, Src0, Src1, C0, C1, C2, relu, lower, _has_src1
    from concourse.dve_uop import DveOpSpec

    name = "EDGE_SCORE_LEAKY_ANT"
    u = Src1 + C0
    body = Src0 * (u * C1 + relu(u) * C2)

    def _ref(in0, in1, c0, c1, c2):
        z = in1.astype(np.float32) + c0
        return in0.astype(np.float32) * (z * c1 + np.maximum(z, 0.0) * c2)

    spec = Spec(body=body, reference=_ref)
    row = dve_ops._CUSTOM_DVE_ROW_BASE + len(dve_ops.OPS)
    shas = {}
    for ver in ("v3", "v4"):
        s = DveOpSpec(name=name, opcode=row, uops=lower(spec, ver=ver),
                      rd1_en=_has_src1(spec))
        shas[ver] = s.sha(ver)
    op = dve_ops.DveOp(name, spec, subdim=False, uops_sha=shas)
    dve_ops.OPS.append(op)
    dve_ops.CUSTOM_DVE_SPECS[name] = spec
    dve_ops._SUB_OPCODE_FOR_NAME[name] = row
    _EDGE_OP = op
    return op


# ---------------------------------------------------------------- device kernel
def build_nc(n=N, n_cores=N_CORES, repeat=1):
    """Build + compile the SPMD program (identical on every core)."""
    edge_op = _register_edge_op()

    r = n // n_cores          # rows of e per core
    jt_n = n // 128           # j tiles (partitions dim)
    lc_n = r // 128           # local row chunks
    cchunks = F_IN // 128     # contraction chunks for HW (2)

    nc = bacc.Bacc("TRN2", target_bir_lowering=False, debug=False,
                   num_devices=n_cores)

    a_slab = nc.dram_tensor("a_slab", [128, jt_n * r], U8, kind="ExternalInput").ap()
    feat_t = nc.dram_tensor("feat_t", [F_IN, n], BF16, kind="ExternalInput").ap()
    featl = nc.dram_tensor("featl", [F_IN, r], BF16, kind="ExternalInput").ap()
    w_sb_in = nc.dram_tensor("w_sb", [128, cchunks, F_OUT], BF16, kind="ExternalInput").ap()
    wt_in = nc.dram_tensor("wt", [F_OUT, F_IN], F32, kind="ExternalInput").ap()
    a2_in = nc.dram_tensor("a2", [F_OUT, 2], F32, kind="ExternalInput").ap()
    ident_in = nc.dram_tensor("ident", [128, 128], F32, kind="ExternalInput").ap()
    yt = nc.dram_tensor("yt", [F_OUT, r], F32, kind="ExternalOutput").ap()

    # DRAM intermediates
    s_stage = nc.dram_tensor("s_stage", [lc_n, 128], F32)
    cc_in = nc.dram_tensor("cc_in", [128, jt_n], F32)
    cc_out = nc.dram_tensor("cc_out", [128, jt_n], F32, addr_space="Shared")

    with tile.TileContext(nc) as tc:
        with ExitStack() as ctx:
            consts = ctx.enter_context(tc.tile_pool(name="consts", bufs=1))
            big = ctx.enter_context(tc.tile_pool(name="big", bufs=1))
            ftp = ctx.enter_context(tc.tile_pool(name="ftp", bufs=4))
            sqp = ctx.enter_context(tc.tile_pool(name="sqp", bufs=3))
            outp = ctx.enter_context(tc.tile_pool(name="outp", bufs=2))
            smallp = ctx.enter_context(tc.tile_pool(name="smallp", bufs=2))
            psS = ctx.enter_context(tc.tile_pool(name="psS", bufs=2, space="PSUM"))
            psH = ctx.enter_context(tc.tile_pool(name="psH", bufs=2, space="PSUM"))
            psO = ctx.enter_context(tc.tile_pool(name="psO", bufs=1, space="PSUM"))

            # constants loaded once
            w_sb = consts.tile([128, cchunks, F_OUT], BF16)
            nc.sync.dma_start(out=w_sb[:], in_=w_sb_in[:])
            wt_sb = consts.tile([F_OUT, F_IN], F32)
            nc.sync.dma_start(out=wt_sb[:], in_=wt_in[:])
            a2_sb = consts.tile([F_OUT, 2], F32)
            nc.sync.dma_start(out=a2_sb[:], in_=a2_in[:])
            ident = consts.tile([128, 128], F32)
            nc.sync.dma_start(out=ident[:], in_=ident_in[:])
            ftl = consts.tile([128, cchunks, r], BF16)
            for c in range(cchunks):
                nc.sync.dma_start(out=ftl[:, c, :], in_=featl[c * 128:(c + 1) * 128, :])

            for rep in range(repeat):
                tag = f"r{rep}"
                # persistent per-iteration tensors
                e_sb = big.tile([128, jt_n, r], BF16, tag="e_sb")
                hw_sb = big.tile([128, jt_n, F_OUT], BF16, tag="hw_sb")
                st_sb = big.tile([128, jt_n, 2], F32, tag="st_sb")
                cn_acc = big.tile([128, jt_n], F32, tag="cn_acc")
                s_cols = big.tile([128, lc_n], F32, tag="s_cols")
                s_rep = big.tile([128, r], BF16, tag="s_rep")
                wa_sb = big.tile([128, cchunks, 2], BF16, tag="wa_sb")

                # ---- a_slab -> e_sb (uint8 -> bf16 cast in DMA), 4 tiles per DMA
                GRP = 4
                for g in range(jt_n // GRP):
                    sl = slice(g * GRP * r, (g + 1) * GRP * r)
                    nc.gpsimd.dma_start(
                        out=e_sb.rearrange("p j r -> p (j r)")[:, sl],
                        in_=a_slab[:, sl])

                # ---- wa = [W @ a1, W @ a2]  (f32 matmul, tiny)
                for c in range(cchunks):
                    wa_ps = psS.tile([128, 2], F32, tag="ps_small")
                    nc.tensor.matmul(wa_ps[:], lhsT=wt_sb[:, c * 128:(c + 1) * 128],
                                     rhs=a2_sb[:], start=True, stop=True)
                    nc.vector.tensor_copy(wa_sb[:, c, :], wa_ps[:])

                # ---- HW + st per j-tile (feat_t streamed in groups of 8 tiles)
                FT_G = 8
                for g in range(jt_n // FT_G):
                    fts = []
                    for c in range(cchunks):
                        ft = ftp.tile([128, FT_G * 128], BF16, tag="ft")
                        nc.sync.dma_start(
                            out=ft[:],
                            in_=feat_t[c * 128:(c + 1) * 128,
                                       g * FT_G * 128:(g + 1) * FT_G * 128])
                        fts.append(ft)
                    for j in range(FT_G):
                        jt = g * FT_G + j
                        off = j * 128
                        hw_ps = psH.tile([128, F_OUT], F32, tag="hw_ps")
                        st_ps = psS.tile([128, 2], F32, tag="ps_small")
                        for c in range(cchunks):
                            nc.tensor.matmul(hw_ps[:], lhsT=fts[c][:, off:off + 128],
                                             rhs=w_sb[:, c, :],
                                             start=(c == 0), stop=(c == cchunks - 1))
                            nc.tensor.matmul(st_ps[:], lhsT=fts[c][:, off:off + 128],
                                             rhs=wa_sb[:, c, :],
                                             start=(c == 0), stop=(c == cchunks - 1))
                        nc.vector.tensor_copy(hw_sb[:, jt, :], hw_ps[:])
                        nc.vector.tensor_copy(st_sb[:, jt, :], st_ps[:])

                # ---- s for local rows -> s_cols [128, lc_n]
                for lc in range(lc_n):
                    sl_ps = psS.tile([128, 1], F32, tag="ps_small")
                    for c in range(cchunks):
                        nc.tensor.matmul(sl_ps[:],
                                         lhsT=ftl[:, c, lc * 128:(lc + 1) * 128],
                                         rhs=wa_sb[:, c, 0:1],
                                         start=(c == 0), stop=(c == cchunks - 1))
                    nc.vector.tensor_copy(s_cols[:, lc:lc + 1], sl_ps[:])

                # ---- transpose s_cols -> DRAM -> broadcast to s_rep (bf16)
                tr_ps = psS.tile([lc_n, 128], F32, tag="ps_small")
                nc.tensor.transpose(tr_ps[:], s_cols[:], ident[:])
                tr_sb = smallp.tile([lc_n, 128], F32, tag="tr_sb")
                nc.vector.tensor_copy(tr_sb[:], tr_ps[:])
                nc.sync.dma_start(out=s_stage.ap(), in_=tr_sb[:])
                s_flat = s_stage.ap().rearrange("l p -> (l p)")
                s_bcast = bass.AP(tensor=s_flat.tensor, offset=s_flat.offset,
                                  ap=[[0, 128]] + s_flat.ap)
                nc.gpsimd.dma_start(out=s_rep[:], in_=s_bcast)

                # ---- phase A: e = A * leaky(s + t); cn2 partial via ACT square
                for jt in range(jt_n):
                    e_sl = e_sb[:, jt, :]
                    nc.vector._custom_dve(
                        edge_op, out=e_sl, in0=e_sl, in1=s_rep[:],
                        s0=st_sb[:, jt, 1:2], s1=NEG_SLOPE, imm2=1.0 - NEG_SLOPE)
                    sqt = sqp.tile([128, r], BF16, tag="sqt")
                    nc.scalar.activation(sqt[:], e_sl,
                                         mybir.ActivationFunctionType.Square,
                                         accum_out=cn_acc[:, jt:jt + 1])

                # ---- AllReduce cn2 partials
                nc.sync.dma_start(out=cc_in.ap(), in_=cn_acc[:])
                nc.gpsimd.collective_compute(
                    "AllReduce", mybir.AluOpType.add,
                    replica_groups=[list(range(n_cores))],
                    ins=[cc_in.ap()], outs=[cc_out.ap()])
                cn2 = smallp.tile([128, jt_n], F32, tag="cn2")
                nc.sync.dma_start(out=cn2[:], in_=cc_out.ap())

                # inv_cn = 1 / max(sqrt(cn2), eps)
                cnr = smallp.tile([128, jt_n], F32, tag="cnr")
                nc.scalar.activation(cnr[:], cn2[:], mybir.ActivationFunctionType.Sqrt)
                nc.vector.tensor_scalar_max(cnr[:], cnr[:], NORM_EPS)
                inv_cn = smallp.tile([128, jt_n], F32, tag="inv_cn")
                nc.vector.reciprocal(inv_cn[:], cnr[:])

                # ---- G = HW * inv_cn (per-partition scalar, in place)
                for jt in range(jt_n):
                    nc.vector.tensor_scalar_mul(hw_sb[:, jt, :], hw_sb[:, jt, :],
                                                inv_cn[:, jt:jt + 1])

                # ---- phase B: yt = G^T @ e  ([F_OUT, r], G stationary per tile)
                o_ps = psO.tile([F_OUT, r], F32, tag="o_ps")
                nhalf = (r + 511) // 512
                for jt in range(jt_n):
                    for h in range(nhalf):
                        sl = slice(h * 512, min((h + 1) * 512, r))
                        nc.tensor.matmul(o_ps[:, sl], lhsT=hw_sb[:, jt, :],
                                         rhs=e_sb[:, jt, sl],
                                         start=(jt == 0), stop=(jt == jt_n - 1))
                for h in range(nhalf):
                    sl = slice(h * 512, min((h + 1) * 512, r))
                    out_t = outp.tile([F_OUT, 512], F32, tag="out_t")
                    w = sl.stop - sl.start
                    nc.vector.tensor_copy(out_t[:, :w], o_ps[:, sl])
                    nc.sync.dma_start(out=yt[:, sl], in_=out_t[:, :w])

    nc.compile()
    return nc


# ---------------------------------------------------------------- host wrapper
def prep_inputs(features, W, a, edge_index, n=N, n_cores=N_CORES):
    features = np.asarray(features, np.float32)
    W = np.asarray(W, np.float32)
    a = np.asarray(a, np.float32).reshape(-1)
    ei = np.asarray(edge_index).astype(np.int64)
    r = n // n_cores
    jt_n = n // 128
    cchunks = F_IN // 128

    # dense adjacency counts, transposed: at[j, i] = #edges (i -> j)
    flat = ei[1] * n + ei[0]
    counts = np.bincount(flat, minlength=n * n)
    assert counts.max() < 256
    at = counts.astype(np.uint8).reshape(n, n)

    feat_t = np.ascontiguousarray(features.T).astype(ml_dtypes.bfloat16)
    w_sb = np.ascontiguousarray(
        W.reshape(cchunks, 128, F_OUT).transpose(1, 0, 2)).astype(ml_dtypes.bfloat16)
    wt = np.ascontiguousarray(W.T).astype(np.float32)
    a2 = np.ascontiguousarray(np.stack([a[:F_OUT], a[F_OUT:]], axis=1)).astype(np.float32)
    ident = np.eye(128, dtype=np.float32)

    in_maps = []
    for m in range(n_cores):
        sl = at[:, m * r:(m + 1) * r]
        a_slab = np.ascontiguousarray(
            sl.reshape(jt_n, 128, r).transpose(1, 0, 2).reshape(128, jt_n * r))
        featl = np.ascontiguousarray(feat_t[:, m * r:(m + 1) * r])
        in_maps.append(dict(a_slab=a_slab, feat_t=feat_t, featl=featl,
                            w_sb=w_sb, wt=wt, a2=a2, ident=ident))
    return in_maps


def assemble_output(results, n=N, n_cores=N_CORES):
    r = n // n_cores
    out = np.empty((n, F_OUT), np.float32)
    for m in range(n_cores):
        out[m * r:(m + 1) * r] = results[m]["yt"].T
    return out


_NC_CACHE = {}


def kernel(features, W, a, edge_index):
    key = (N, N_CORES, 1)
    if key not in _NC_CACHE:
        _NC_CACHE[key] = build_nc(N, N_CORES, repeat=1)
    nc = _NC_CACHE[key]
    in_maps = prep_inputs(features, W, a, edge_index)
    res = bass_utils.run_bass_kernel_spmd(nc, in_maps, core_ids=list(range(N_CORES)))
    return assemble_output(res.results)
